# revision 1
# baseline (speedup 1.0000x reference)
"""Trainium2 Bass kernel for nn_AttentionBlock (GroupNorm -> QKV -> 4-head
attention over 4096 seq -> proj -> residual), B=4, C=256, H=W=64.

Sharding: 8 cores = (batch b, sequence-half). Each core redundantly computes
GroupNorm + K/V for its batch (cheap), and Q/attention/proj for its half of
the sequence. No collectives. The host rotates x's spatial columns per core
so a single SPMD program serves all 8 cores (attention output is invariant
to key/value ordering; queries are always columns 0:2048 of the rotated x).

Attention is computed in transposed layout T = S^T[sk, sq] so the softmax
denominator and the PV contraction both run on the PE:
  - T chunk = matmul(lhsT=k[d, sk_chunk], rhs=q[d, sq_block])  (contraction d=64,
    the two heads of a 128-channel pair row-packed at tile_position (0,0)/(64,0))
  - P^T = exp(T * 1/8) on ScalarE straight from multi-bank PSUM (no max
    subtraction needed: fp32 exp cannot overflow for logits this size)
  - O^T[d, sq] (+ denominator row via a ones-column appended to V^T) accumulate
    over the 32 sk chunks in PSUM
  - normalize: denominator row bounced through DRAM to broadcast across
    partitions (stride-0 partition APs are DRAM-only), fast approximate
    reciprocal + multiply on DVE.

The attention inner loop is software-pipelined (QK of unit u+1 is emitted
before PV of unit u) over a 3-deep shared ring of 2-bank PSUM T tiles, so the
PE always has ready work while ScalarE's exp (the roofline: ~33.5M exps/core
at 1 elem/cycle/lane) drains the previous tile. Measured ~547us/core/iter on
HW; ScalarE-exp floor is ~280us.
"""

import math
import numpy as np

import concourse.bacc as bacc
import concourse.bass as bass
import concourse.tile as tile
from concourse import mybir
from concourse.alu_op_type import AluOpType
from concourse.bass_utils import run_bass_kernel_spmd

B, C, S = 4, 256, 4096
NH, D, G = 4, 64, 8
EPS = 1e-5
SQ = S // 2          # 2048 queries per core
NCK = S // 128       # 32 key chunks
FP = mybir.dt.float32
FR = mybir.dt.float32r
BF = mybir.dt.bfloat16
AF = mybir.ActivationFunctionType

# sk chunk pairs: T tiles of 2 psum banks (exp call N=1024), 3-deep ring
TRIS = [(i, min(2, NCK - i)) for i in range(0, NCK, 2)]


def _emit_prefix(nc, tc, sb, ps, dr, dram):
    x_in, wqkvT, wprojT, bqk, gnw, gnb, pbeff, oneg, onegT, out_d = dram

    # ---------------- constants / weights ----------------
    c_oneg = sb.tile([128, 4], FP, tag="cg")
    nc.sync.dma_start(out=c_oneg, in_=oneg[:, :])
    c_onegT = sb.tile([4, 128], FP, tag="cgt")
    nc.sync.dma_start(out=c_onegT, in_=onegT[:, :])
    ones_sb = sb.tile([128, 64], FP, tag="ones")
    nc.vector.memset(ones_sb, 1.0)

    w0 = sb.tile([128, 3 * C], FR, tag="W", bufs=2)
    w1 = sb.tile([128, 3 * C], FR, tag="W", bufs=2)
    nc.sync.dma_start(out=w0, in_=wqkvT[0:128, :])
    nc.sync.dma_start(out=w1, in_=wqkvT[128:256, :])
    wp0 = sb.tile([128, C], FR, tag="WP", bufs=2)
    wp1 = sb.tile([128, C], FR, tag="WP", bufs=2)
    nc.sync.dma_start(out=wp0, in_=wprojT[0:128, :])
    nc.sync.dma_start(out=wp1, in_=wprojT[128:256, :])

    def vec(name):
        return sb.tile([128, 1], FP, tag="vecs", bufs=12, name=name)

    gnw_t, gnb_t, pb_t, bq_t, bk_t = [], [], [], [], []
    for t in (0, 1):
        gw = vec(f"gnw{t}"); nc.sync.dma_start(out=gw, in_=gnw[t * 128:(t + 1) * 128, :]); gnw_t.append(gw)
        gb = vec(f"gnb{t}"); nc.sync.dma_start(out=gb, in_=gnb[t * 128:(t + 1) * 128, :]); gnb_t.append(gb)
        pb = vec(f"pbe{t}"); nc.sync.dma_start(out=pb, in_=pbeff[t * 128:(t + 1) * 128, :]); pb_t.append(pb)
        bq = vec(f"bq{t}"); nc.sync.dma_start(out=bq, in_=bqk[t * 128:(t + 1) * 128, :]); bq_t.append(bq)
        bk = vec(f"bk{t}"); nc.sync.dma_start(out=bk, in_=bqk[256 + t * 128:256 + (t + 1) * 128, :]); bk_t.append(bk)

    # round-robin psum tags for the matmul-evac phases (pre/post attention)
    _rr = [0]
    PS_TAGS = ["T", "O0", "O1"]
    def rr_ps(shape):
        tag = PS_TAGS[_rr[0] % 3]
        _rr[0] += 1
        return ps.tile(shape, FP, tag=tag, name=f"mm{_rr[0]}", bufs=3 if tag == "T" else 1)

    # ---------------- phase 1: load x + GroupNorm ----------------
    xns = []
    for t in (0, 1):
        x_t = sb.tile([128, S], FP, tag="BIG", bufs=2)
        for c8 in range(8):
            nc.sync.dma_start(out=x_t[:, c8 * 512:(c8 + 1) * 512],
                              in_=x_in[t * 128:(t + 1) * 128, c8 * 512:(c8 + 1) * 512])
        st = sb.tile([128, 8, 6], FP, tag="bn", bufs=2)
        for c8 in range(8):
            nc.vector.bn_stats(out=st[:, c8, :], in_=x_t[:, c8 * 512:(c8 + 1) * 512])
        mv = sb.tile([128, 2], FP, tag="mv", bufs=2)
        nc.vector.bn_aggr(out=mv, in_=st)
        # per-partition [mean, E[x^2]]
        s2 = sb.tile([128, 2], FP, tag="s2", bufs=2)
        nc.vector.tensor_copy(s2[:, 0:1], mv[:, 0:1])
        nc.vector.tensor_tensor(s2[:, 1:2], mv[:, 0:1], mv[:, 0:1], op=AluOpType.mult)
        nc.vector.tensor_tensor(s2[:, 1:2], s2[:, 1:2], mv[:, 1:2], op=AluOpType.add)
        # reduce over the 32 partitions of each group (4 groups per 128-chunk)
        gps = rr_ps([4, 2])
        nc.tensor.matmul(gps, lhsT=c_oneg, rhs=s2, start=True, stop=True)
        gsb = sb.tile([4, 2], FP, tag="gsb", bufs=2)
        nc.vector.tensor_scalar_mul(gsb, gps, 1.0 / 32.0)
        tmp4 = sb.tile([4, 1], FP, tag="t4", bufs=2)
        nc.vector.tensor_tensor(tmp4, gsb[:, 0:1], gsb[:, 0:1], op=AluOpType.mult)
        nc.vector.tensor_tensor(gsb[:, 1:2], gsb[:, 1:2], tmp4, op=AluOpType.subtract)
        epsb = sb.tile([4, 1], FP, tag="eps", bufs=2, name=f"eps{t}")
        nc.vector.memset(epsb, EPS)
        nc.scalar.activation(gsb[:, 1:2], gsb[:, 1:2], AF.Sqrt, bias=epsb)
        nc.vector.reciprocal(gsb[:, 1:2], gsb[:, 1:2])       # rstd = 1/sqrt(var+eps)
        # broadcast group stats back to the 128 channel partitions
        bc = rr_ps([128, 2])
        nc.tensor.matmul(bc, lhsT=c_onegT, rhs=gsb, start=True, stop=True)
        scl = sb.tile([128, 1], FP, tag="scl", bufs=2)
        sft = sb.tile([128, 1], FP, tag="sft", bufs=2)
        nc.vector.tensor_tensor(scl, bc[:, 1:2], gnw_t[t], op=AluOpType.mult)
        nc.vector.tensor_tensor(sft, bc[:, 0:1], scl, op=AluOpType.mult)
        nc.vector.tensor_tensor(sft, gnb_t[t], sft, op=AluOpType.subtract)
        xn_t = sb.tile([128, S], FR, tag="MID", bufs=2)
        nc.vector.tensor_scalar(xn_t, x_t, scalar1=scl, scalar2=sft,
                                op0=AluOpType.mult, op1=AluOpType.add)
        xns.append(xn_t)

    # ---------------- phase 2: QKV projections ----------------
    # q[qch, sq] for this core's queries; k[kch, sk] full; v^T[sk, (h, d|1)] full
    def emit_q(P, q_t, sqb):
        mm = rr_ps([128, 512])
        nc.tensor.matmul(mm, lhsT=w0[:, P * 128:(P + 1) * 128],
                         rhs=xns[0][:, sqb * 512:(sqb + 1) * 512], start=True, stop=False)
        nc.tensor.matmul(mm, lhsT=w1[:, P * 128:(P + 1) * 128],
                         rhs=xns[1][:, sqb * 512:(sqb + 1) * 512], start=False, stop=True)
        nc.scalar.activation(q_t[:, sqb * 512:(sqb + 1) * 512], mm, AF.Identity, bias=bq_t[P])

    def emit_k(P, k_t, skb):
        mm = rr_ps([128, 512])
        nc.tensor.matmul(mm, lhsT=w0[:, 256 + P * 128:256 + (P + 1) * 128],
                         rhs=xns[0][:, skb * 512:(skb + 1) * 512], start=True, stop=False)
        nc.tensor.matmul(mm, lhsT=w1[:, 256 + P * 128:256 + (P + 1) * 128],
                         rhs=xns[1][:, skb * 512:(skb + 1) * 512], start=False, stop=True)
        nc.scalar.activation(k_t[:, skb * 512:(skb + 1) * 512], mm, AF.Identity, bias=bk_t[P])

    def emit_vt(ck):
        mm = rr_ps([128, 256])
        nc.tensor.matmul(mm, lhsT=xns[0][:, ck * 128:(ck + 1) * 128],
                         rhs=w0[:, 512:768], start=True, stop=False)
        nc.tensor.matmul(mm, lhsT=xns[1][:, ck * 128:(ck + 1) * 128],
                         rhs=w1[:, 512:768], start=False, stop=True)
        vt = sb.tile([128, 4, 65], FR, tag="VT", bufs=NCK, name=f"vt{ck}")
        nc.vector.tensor_copy(vt[:, :, 0:64], mm.rearrange("p (h u) -> p h u", u=64))
        nc.vector.tensor_copy(vt[:, :, 64:65], ones_sb[:, 0:1].to_broadcast((128, 4, 1)))
        return vt

    # emission order: unblock pair-0 attention ASAP (q P0 + interleaved k P0/vT),
    # then pair-1 q/k (not needed until ~half-way through attention)
    qp = [sb.tile([128, SQ], FR, tag="Q", bufs=2, name=f"q{P}") for P in (0, 1)]
    kp = [sb.tile([128, S], FR, tag="K", bufs=2, name=f"k{P}") for P in (0, 1)]
    vts = [None] * NCK
    emit_q(0, qp[0], 0)
    for skb in range(8):
        emit_k(0, kp[0], skb)
        for ck in range(4 * skb, 4 * skb + 4):
            vts[ck] = emit_vt(ck)
    for sqb in range(1, 4):
        emit_q(0, qp[0], sqb)
    for sqb in range(4):
        emit_q(1, qp[1], sqb)
    for skb in range(8):
        emit_k(1, kp[1], skb)

    return dict(qp=qp, kp=kp, vts=vts, x_in=x_in, pb_t=pb_t,
                ones_sb=ones_sb, wp0=wp0, wp1=wp1, out_d=out_d)


def _emit_attn(nc, tc, sb, ps, dr, ctx, dbg=None):
    qp, kp, vts, pb_t = ctx["qp"], ctx["kp"], ctx["vts"], ctx["pb_t"]
    ones_sb, wp0, wp1, out_d = ctx["ones_sb"], ctx["wp0"], ctx["wp1"], ctx["out_d"]
    x_res_dram = ctx["x_in"]

    _rr = [0]
    PS_TAGS = ["T", "O0", "O1"]
    def rr_ps(shape):
        tag = PS_TAGS[_rr[0] % 3]
        _rr[0] += 1
        return ps.tile(shape, FP, tag=tag, name=f"amm{_rr[0]}", bufs=3 if tag == "T" else 1)

    # ---------------- phase 3: attention ----------------
    o_sb = [sb.tile([128, SQ], FR, tag="MID", bufs=2, name=f"osb{i}") for i in (0, 1)]
    for P in (0, 1):
        for sqb in range(4):
            O_ps = [ps.tile([65, 512], FP, tag="O0", name=f"O0_{P}_{sqb}"),
                    ps.tile([65, 512], FP, tag="O1", name=f"O1_{P}_{sqb}")]
            def emit_pv(u):
                uPT, uc0, untri, uhg, uO = u
                for j in range(untri):
                    ck = uc0 + j
                    nc.tensor.matmul(
                        uO,
                        lhsT=vts[ck][:, uhg, :],
                        rhs=uPT[:, j * 512:(j + 1) * 512],
                        start=(ck == 0), stop=(ck == NCK - 1))

            # software-pipelined: emit QK+exp of unit u, then the PV of unit
            # u-1, so the PE always has work while ScalarE runs exp
            pending = None
            for (c0, ntri) in TRIS:
                for h in (0, 1):
                    hg = 2 * P + h
                    T = ps.tile([128, ntri * 512], FP, tag="T", bufs=3)
                    for j in range(ntri):
                        ck = c0 + j
                        nc.tensor.matmul(
                            T[:, j * 512:(j + 1) * 512],
                            lhsT=kp[P][64 * h:64 * (h + 1), ck * 128:(ck + 1) * 128],
                            rhs=qp[P][64 * h:64 * (h + 1), sqb * 512:(sqb + 1) * 512],
                            start=True, stop=True, tile_position=(64 * h, 0))
                    PT = sb.tile([128, ntri * 512], FR, tag="PT", bufs=4)
                    nc.scalar.activation(PT, T, AF.Exp, scale=0.125)
                    if dbg is not None and P == 0 and sqb == 0 and c0 == 0:
                        tdump = sb.tile([128, ntri * 512], FP, tag="OC", bufs=2,
                                        name=f"tdump{h}")
                        nc.vector.tensor_copy(tdump, T)
                        nc.sync.dma_start(out=dbg[f"dT{h}"][:, :], in_=tdump)
                        nc.sync.dma_start(out=dbg[f"dPT{h}"][:, :], in_=PT)
                    if pending is not None:
                        emit_pv(pending)
                    pending = (PT, c0, ntri, hg, O_ps[h])
            emit_pv(pending)
            pending = None
            for h in (0, 1):
                oc = sb.tile([65, 512], FP, tag="OC", bufs=2)
                nc.vector.tensor_copy(oc, O_ps[h])
                if dbg is not None and P == 0 and sqb == 0:
                    nc.sync.dma_start(out=dbg[f"doc{h}"][:, :], in_=oc)
                # broadcast the denominator row across 64 partitions: bounce
                # through DRAM (partition-stride-0 APs are DRAM-only), then
                # reciprocal on DVE
                dscr = dr.tile([1, 512], FP, tag="DSC", bufs=4, name=f"dsc{P}{sqb}{h}")
                nc.sync.dma_start(out=dscr, in_=oc[65 - 1:65, :])
                rbc = sb.tile([64, 512], FP, tag="RBC", bufs=2)
                den_bcast = bass.AP(tensor=dscr.tensor, offset=dscr.offset,
                                    ap=[[0, 64], [1, 512]])
                nc.sync.dma_start(out=rbc, in_=den_bcast)
                nc.vector.reciprocal_approx_fast(rbc, rbc)
                if dbg is not None and P == 0 and sqb == 0:
                    nc.sync.dma_start(out=dbg[f"drbc{h}"][:, :], in_=rbc)
                if h == 0:
                    nc.vector.tensor_tensor(o_sb[P][0:64, sqb * 512:(sqb + 1) * 512],
                                            oc[0:64, :], rbc, op=AluOpType.mult)
                else:
                    tmp = sb.tile([64, 512], FR, tag="TMP", bufs=2)
                    nc.vector.tensor_tensor(tmp, oc[0:64, :], rbc, op=AluOpType.mult)
                    # shift to partitions 64..127 via SBUF->SBUF DMA
                    nc.sync.dma_start(out=o_sb[P][64:128, sqb * 512:(sqb + 1) * 512], in_=tmp)

    if dbg is not None:
        nc.sync.dma_start(out=dbg["dq0"][:, :], in_=qp[0])
        nc.sync.dma_start(out=dbg["dk0"][:, :], in_=kp[0])
        nc.sync.dma_start(out=dbg["dxn0"][:, :], in_=xns[0])
        nc.sync.dma_start(out=dbg["dvt0"][:, :], in_=vts[0].rearrange("p a b -> p (a b)"))
        nc.sync.dma_start(out=dbg["dosb0"][:, :], in_=o_sb[0])
        nc.sync.dma_start(out=dbg["dosb1"][:, :], in_=o_sb[1])

    # ---------------- phase 4: projection + residual ----------------
    for och in (0, 1):
        for sqb in range(4):
            pp = rr_ps([128, 512])
            nc.tensor.matmul(pp, lhsT=wp0[:, och * 128:(och + 1) * 128],
                             rhs=o_sb[0][:, sqb * 512:(sqb + 1) * 512], start=True, stop=False)
            nc.tensor.matmul(pp, lhsT=wp1[:, och * 128:(och + 1) * 128],
                             rhs=o_sb[1][:, sqb * 512:(sqb + 1) * 512], start=False, stop=True)
            fin = sb.tile([128, 512], FP, tag="FIN", bufs=4)
            xrs = sb.tile([128, 512], FP, tag="XR2", bufs=4, name=f"xr{och}{sqb}")
            nc.sync.dma_start(out=xrs, in_=x_res_dram[och * 128:(och + 1) * 128,
                                                      sqb * 512:(sqb + 1) * 512])
            nc.vector.scalar_tensor_tensor(fin, in0=pp, scalar=pb_t[och],
                                           in1=xrs,
                                           op0=AluOpType.add, op1=AluOpType.add)
            nc.sync.dma_start(out=out_d[och * 128:(och + 1) * 128, sqb * 512:(sqb + 1) * 512],
                              in_=fin)



def _emit_body(nc, tc, sb, ps, dr, dram, dbg=None):
    ctx = _emit_prefix(nc, tc, sb, ps, dr, dram)
    _emit_attn(nc, tc, sb, ps, dr, ctx, dbg=dbg)


def build_program(loop_n: int = 1, debug: bool = False, timing: bool = False, loop_part: str = "all"):
    nc = bacc.Bacc("TRN2", target_bir_lowering=False)
    if timing:
        # Timing-only build: identical device work, but all big tensors are
        # internal DRAM (garbage contents — every op here is data-independent
        # in latency), so each dispatch ships only a 4-byte token instead of
        # ~7MB/core. Wall-clock deltas then resolve the kernel time.
        tok_in = nc.dram_tensor("tok", [1, 1], FP, kind="ExternalInput")
        tok_out = nc.dram_tensor("tok_out", [1, 1], FP, kind="ExternalOutput")
        kind = "Internal"
    else:
        kind = "ExternalInput"
    x_in = nc.dram_tensor("x", [C, S], FP, kind=kind)
    wqkvT = nc.dram_tensor("wqkvT", [C, 3 * C], FR, kind=kind)
    wprojT = nc.dram_tensor("wprojT", [C, C], FR, kind=kind)
    bqk = nc.dram_tensor("bqk", [2 * C, 1], FP, kind=kind)
    gnw = nc.dram_tensor("gnw", [C, 1], FP, kind=kind)
    gnb = nc.dram_tensor("gnb", [C, 1], FP, kind=kind)
    pbeff = nc.dram_tensor("pbeff", [C, 1], FP, kind=kind)
    oneg = nc.dram_tensor("oneg", [128, 4], FP, kind=kind)
    onegT = nc.dram_tensor("onegT", [4, 128], FP, kind=kind)
    out_d = nc.dram_tensor("out", [C, SQ], FP,
                           kind="Internal" if timing else "ExternalOutput")
    dram = (x_in, wqkvT, wprojT, bqk, gnw, gnb, pbeff, oneg, onegT, out_d)

    dbg = None
    if debug:
        dbg = {}
        for name, shape, dt_ in [("dT0", [128, 1536], FP), ("dT1", [128, 1536], FP),
                                 ("dPT0", [128, 1536], FR), ("dPT1", [128, 1536], FR),
                                 ("doc0", [65, 512], FP), ("doc1", [65, 512], FP),
                                 ("dq0", [128, SQ], FR), ("dk0", [128, S], FR),
                                 ("dxn0", [128, S], FR), ("dvt0", [128, 260], FR),
                                 ("dosb0", [128, SQ], FR), ("dosb1", [128, SQ], FR),
                                 ("drbc0", [64, 512], FP), ("drbc1", [64, 512], FP)]:
            dbg[name] = nc.dram_tensor(name, shape, dt_, kind="ExternalOutput")

    with tile.TileContext(nc) as tc:
        with (tc.tile_pool(name="sb", bufs=1) as sb,
              tc.tile_pool(name="ps", bufs=1, space="PSUM") as ps,
              tc.tile_pool(name="dr", bufs=1, space="DRAM") as dr):
            if timing:
                tokt = sb.tile([1, 1], FP, tag="tok")
                nc.sync.dma_start(out=tokt, in_=tok_in[:, :])
                nc.sync.dma_start(out=tok_out[:, :], in_=tokt)
            # psum tag slots: T0/T1 3 banks each, O0/O1 1 bank each = 8 banks
            if loop_n == 1:
                _emit_body(nc, tc, sb, ps, dr, dram, dbg=dbg)
            elif loop_part == "all":
                with tc.For_i(0, loop_n, 1):
                    _emit_body(nc, tc, sb, ps, dr, dram)
            elif loop_part == "prefix":
                with tc.For_i(0, loop_n, 1):
                    _emit_prefix(nc, tc, sb, ps, dr, dram)
            elif loop_part == "attn":
                ctx = _emit_prefix(nc, tc, sb, ps, dr, dram)
                with tc.For_i(0, loop_n, 1):
                    _emit_attn(nc, tc, sb, ps, dr, ctx)
            else:
                raise ValueError(loop_part)
    nc.finalize()
    return nc


def make_in_maps(inputs):
    x = np.asarray(inputs["x"], np.float32).reshape(B, C, S)
    gn_w = np.asarray(inputs["gn_w"], np.float32)
    gn_b = np.asarray(inputs["gn_b"], np.float32)
    qkv_w = np.asarray(inputs["qkv_w"], np.float32)
    qkv_b = np.asarray(inputs["qkv_b"], np.float32)
    proj_w = np.asarray(inputs["proj_w"], np.float32)
    proj_b = np.asarray(inputs["proj_b"], np.float32)

    wqkvT = np.ascontiguousarray(qkv_w.T)                      # [C, 3C]
    wprojT = np.ascontiguousarray(proj_w.T)                    # [C, C]
    bqk = np.ascontiguousarray(qkv_b[:2 * C].reshape(2 * C, 1))
    bv = qkv_b[2 * C:]
    pbeff = np.ascontiguousarray((proj_w @ bv + proj_b).reshape(C, 1).astype(np.float32))
    gnw2 = np.ascontiguousarray(gn_w.reshape(C, 1))
    gnb2 = np.ascontiguousarray(gn_b.reshape(C, 1))
    oneg = np.zeros((128, 4), np.float32)
    oneg[np.arange(128), np.arange(128) // 32] = 1.0
    onegT = np.ascontiguousarray(oneg.T)

    in_maps = []
    for c in range(8):
        b, half = c // 2, c % 2
        x_rot = np.ascontiguousarray(np.roll(x[b], -half * SQ, axis=1))
        in_maps.append(dict(x=x_rot, wqkvT=wqkvT, wprojT=wprojT, bqk=bqk,
                            gnw=gnw2, gnb=gnb2, pbeff=pbeff, oneg=oneg, onegT=onegT))
    return in_maps


def assemble_output(results):
    out = np.empty((B, C, S), np.float32)
    for c in range(8):
        b, half = c // 2, c % 2
        out[b][:, half * SQ:(half + 1) * SQ] = results[c]["out"]
    return out.reshape(B, C, 64, 64)


_prog_cache = {}


def kernel(**inputs):
    if "nc" not in _prog_cache:
        _prog_cache["nc"] = build_program(loop_n=1)
    nc = _prog_cache["nc"]
    in_maps = make_in_maps(inputs)
    res = run_bass_kernel_spmd(nc, in_maps, core_ids=list(range(8)), trace=False)
    return assemble_output(res.results)



# revision 2
# speedup vs baseline: 1.3555x; 1.3555x over previous
"""Trainium2 Bass kernel for nn_AttentionBlock (GroupNorm -> QKV -> 4-head
attention over 4096 seq -> proj -> residual), B=4, C=256, H=W=64.

Sharding: 8 cores = (batch b, sequence-half). Each core redundantly computes
GroupNorm + K/V for its batch (cheap), and Q/attention/proj for its half of
the sequence. No collectives. The host rotates x's spatial columns per core
so a single SPMD program serves all 8 cores (attention output is invariant
to key/value ordering; queries are always columns 0:2048 of the rotated x).

v2 engine plan (per core, 33.5M softmax exps is the roofline):
  - QKV/QK/proj matmuls in bf16 (PE streams 1 col/cycle @2.4GHz warm; QK
    row-packs the two heads of a pair at tile_position (0,0)/(64,0)).
  - exp is SPLIT across two engines, alternating sk-chunk-pairs:
      ScalarE: true exp LUT -> fp8e4m3 PT   (153.6 G elem/s)
      VectorE: Schraudolph bit-trick -> uint8 -> bitcast fp8e4m3
               (y = round(1.4427*T + B); fp32->uint8 saturates at 0 for
               logit underflow; one 1x tensor_scalar from PSUM, 123 G/s)
    A global logit shift of -4 (exp(x-4)) keeps exp below fp8 max; it
    cancels in the softmax ratio. Both engines use the same shift.
  - PV runs in fp8 DoubleRow (2 contraction rows/cell, 0.5 cyc/row): V is
    stored as [128, (ko=2, h=4, 80pad)] fp8 pair tiles with a ones column
    at d=64 so O_ps[64,:] accumulates the softmax denominator.
  - normalize: denominator row bounced through DRAM to broadcast across
    partitions, reciprocal_approx_fast + multiply on DVE; proj in bf16 and
    residual-add in fp32 as before.
"""

import math
import numpy as np
import ml_dtypes

import concourse.bacc as bacc
import concourse.bass as bass
import concourse.tile as tile
from concourse import mybir
from concourse.alu_op_type import AluOpType
from concourse.bass_utils import run_bass_kernel_spmd

B, C, S = 4, 256, 4096
NH, D, G = 4, 64, 8
EPS = 1e-5
SQ = S // 2          # 2048 queries per core
NCK = S // 128       # 32 key chunks
NPR = NCK // 2       # 16 chunk pairs (DoubleRow contraction = 256)
FP = mybir.dt.float32
BF = mybir.dt.bfloat16
F8 = mybir.dt.float8e4
U8 = mybir.dt.uint8
AF = mybir.ActivationFunctionType
DR = mybir.MatmulPerfMode.DoubleRow

SHIFT = 4.0                       # exp(x - SHIFT); cancels in softmax
# Schraudolph fp8e4m3 trick: byte = round(A8*(0.125*T) + B8)
A8 = 8.0 / math.log(2.0)          # 11.5416 (8 = 2^mantissa_bits)
TS_SCALE1 = 0.125 * A8            # 1.4426950
TS_SCALE2 = -SHIFT * A8 + 7 * 8 + 0.5 - 0.558   # bias 7, mid adjust

# pairs whose exp runs on VectorE (rest on ScalarE): 7/16 = 43.75%
DVE_PAIRS = frozenset({1, 3, 5, 7, 9, 11, 13})


def _emit_prefix(nc, tc, sb, ps, dr, dram):
    x_in, wqkvT, wprojT, bqk, gnw, gnb, pbeff, oneg, onegT, out_d = dram

    # ---------------- constants / weights ----------------
    c_oneg = sb.tile([128, 4], FP, tag="cg")
    nc.sync.dma_start(out=c_oneg, in_=oneg[:, :])
    c_onegT = sb.tile([4, 128], FP, tag="cgt")
    nc.sync.dma_start(out=c_onegT, in_=onegT[:, :])

    w0 = sb.tile([128, 3 * C], BF, tag="W", bufs=2)
    w1 = sb.tile([128, 3 * C], BF, tag="W", bufs=2)
    nc.sync.dma_start(out=w0, in_=wqkvT[0:128, :])
    nc.sync.dma_start(out=w1, in_=wqkvT[128:256, :])
    wp0 = sb.tile([128, C], BF, tag="WP", bufs=2)
    wp1 = sb.tile([128, C], BF, tag="WP", bufs=2)
    nc.sync.dma_start(out=wp0, in_=wprojT[0:128, :])
    nc.sync.dma_start(out=wp1, in_=wprojT[128:256, :])

    def vec(name):
        return sb.tile([128, 1], FP, tag="vecs", bufs=12, name=name)

    gnw_t, gnb_t, pb_t, bq_t, bk_t = [], [], [], [], []
    for t in (0, 1):
        gw = vec(f"gnw{t}"); nc.sync.dma_start(out=gw, in_=gnw[t * 128:(t + 1) * 128, :]); gnw_t.append(gw)
        gb = vec(f"gnb{t}"); nc.sync.dma_start(out=gb, in_=gnb[t * 128:(t + 1) * 128, :]); gnb_t.append(gb)
        pb = vec(f"pbe{t}"); nc.sync.dma_start(out=pb, in_=pbeff[t * 128:(t + 1) * 128, :]); pb_t.append(pb)
        bq = vec(f"bq{t}"); nc.sync.dma_start(out=bq, in_=bqk[t * 128:(t + 1) * 128, :]); bq_t.append(bq)
        bk = vec(f"bk{t}"); nc.sync.dma_start(out=bk, in_=bqk[256 + t * 128:256 + (t + 1) * 128, :]); bk_t.append(bk)
    shift_t = sb.tile([128, 1], FP, tag="vecs", bufs=12, name="shift")
    nc.vector.memset(shift_t, -SHIFT)

    # round-robin psum tags for the matmul-evac phases (pre/post attention)
    _rr = [0]
    PS_TAGS = ["T", "O0", "O1"]
    def rr_ps(shape):
        tag = PS_TAGS[_rr[0] % 3]
        _rr[0] += 1
        return ps.tile(shape, FP, tag=tag, name=f"mm{_rr[0]}", bufs=3 if tag == "T" else 1)

    # ---------------- phase 1: load x + GroupNorm ----------------
    xns = []
    for t in (0, 1):
        x_t = sb.tile([128, S], FP, tag="BIG", bufs=2)
        for c8 in range(8):
            nc.sync.dma_start(out=x_t[:, c8 * 512:(c8 + 1) * 512],
                              in_=x_in[t * 128:(t + 1) * 128, c8 * 512:(c8 + 1) * 512])
        st = sb.tile([128, 8, 6], FP, tag="bn", bufs=2)
        for c8 in range(8):
            nc.vector.bn_stats(out=st[:, c8, :], in_=x_t[:, c8 * 512:(c8 + 1) * 512])
        mv = sb.tile([128, 2], FP, tag="mv", bufs=2)
        nc.vector.bn_aggr(out=mv, in_=st)
        # per-partition [mean, E[x^2]]
        s2 = sb.tile([128, 2], FP, tag="s2", bufs=2)
        nc.vector.tensor_copy(s2[:, 0:1], mv[:, 0:1])
        nc.vector.tensor_tensor(s2[:, 1:2], mv[:, 0:1], mv[:, 0:1], op=AluOpType.mult)
        nc.vector.tensor_tensor(s2[:, 1:2], s2[:, 1:2], mv[:, 1:2], op=AluOpType.add)
        # reduce over the 32 partitions of each group (4 groups per 128-chunk)
        gps = rr_ps([4, 2])
        nc.tensor.matmul(gps, lhsT=c_oneg, rhs=s2, start=True, stop=True)
        gsb = sb.tile([4, 2], FP, tag="gsb", bufs=2)
        nc.vector.tensor_scalar_mul(gsb, gps, 1.0 / 32.0)
        tmp4 = sb.tile([4, 1], FP, tag="t4", bufs=2)
        nc.vector.tensor_tensor(tmp4, gsb[:, 0:1], gsb[:, 0:1], op=AluOpType.mult)
        nc.vector.tensor_tensor(gsb[:, 1:2], gsb[:, 1:2], tmp4, op=AluOpType.subtract)
        epsb = sb.tile([4, 1], FP, tag="eps", bufs=2, name=f"eps{t}")
        nc.vector.memset(epsb, EPS)
        nc.scalar.activation(gsb[:, 1:2], gsb[:, 1:2], AF.Sqrt, bias=epsb)
        nc.vector.reciprocal(gsb[:, 1:2], gsb[:, 1:2])       # rstd = 1/sqrt(var+eps)
        # broadcast group stats back to the 128 channel partitions
        bc = rr_ps([128, 2])
        nc.tensor.matmul(bc, lhsT=c_onegT, rhs=gsb, start=True, stop=True)
        scl = sb.tile([128, 1], FP, tag="scl", bufs=2)
        sft = sb.tile([128, 1], FP, tag="sft", bufs=2)
        nc.vector.tensor_tensor(scl, bc[:, 1:2], gnw_t[t], op=AluOpType.mult)
        nc.vector.tensor_tensor(sft, bc[:, 0:1], scl, op=AluOpType.mult)
        nc.vector.tensor_tensor(sft, gnb_t[t], sft, op=AluOpType.subtract)
        xn_t = sb.tile([128, S], BF, tag="MID", bufs=2)
        nc.vector.tensor_scalar(xn_t, x_t, scalar1=scl, scalar2=sft,
                                op0=AluOpType.mult, op1=AluOpType.add)
        xns.append(xn_t)

    # ---------------- phase 2: QKV projections ----------------
    # q[qch, sq] for this core's queries; k[kch, sk] full (bf16);
    # v as fp8 DoubleRow pair tiles [128, (ko=2, h=4, 80)] with ones col at 64
    def emit_q(P, q_t, sqb):
        mm = rr_ps([128, 512])
        nc.tensor.matmul(mm, lhsT=w0[:, P * 128:(P + 1) * 128],
                         rhs=xns[0][:, sqb * 512:(sqb + 1) * 512], start=True, stop=False)
        nc.tensor.matmul(mm, lhsT=w1[:, P * 128:(P + 1) * 128],
                         rhs=xns[1][:, sqb * 512:(sqb + 1) * 512], start=False, stop=True)
        nc.scalar.activation(q_t[:, sqb * 512:(sqb + 1) * 512], mm, AF.Identity, bias=bq_t[P])

    def emit_k(P, k_t, skb):
        mm = rr_ps([128, 512])
        nc.tensor.matmul(mm, lhsT=w0[:, 256 + P * 128:256 + (P + 1) * 128],
                         rhs=xns[0][:, skb * 512:(skb + 1) * 512], start=True, stop=False)
        nc.tensor.matmul(mm, lhsT=w1[:, 256 + P * 128:256 + (P + 1) * 128],
                         rhs=xns[1][:, skb * 512:(skb + 1) * 512], start=False, stop=True)
        nc.scalar.activation(k_t[:, skb * 512:(skb + 1) * 512], mm, AF.Identity, bias=bk_t[P])

    def emit_vpair(pr):
        # two sk chunks c0=2pr, c1=2pr+1 -> one [128, 512] psum ([ko, h, d])
        mm = rr_ps([128, 512])
        for j in range(2):
            ck = 2 * pr + j
            nc.tensor.matmul(mm[:, j * 256:(j + 1) * 256],
                             lhsT=xns[0][:, ck * 128:(ck + 1) * 128],
                             rhs=w0[:, 512:768], start=True, stop=False)
            nc.tensor.matmul(mm[:, j * 256:(j + 1) * 256],
                             lhsT=xns[1][:, ck * 128:(ck + 1) * 128],
                             rhs=w1[:, 512:768], start=False, stop=True)
        vt = sb.tile([128, 2, 4, 80], F8, tag="VT", bufs=NPR, name=f"vt{pr}")
        nc.scalar.activation(vt[:, :, :, 0:64],
                             mm.rearrange("p (k h u) -> p k h u", k=2, u=64),
                             AF.Identity)
        nc.vector.memset(vt[:, :, :, 64:65], 1.0)
        return vt

    # emission order: unblock pair-0 attention ASAP (q P0 + interleaved k P0/vt),
    # then pair-1 q/k (not needed until ~half-way through attention)
    qp = [sb.tile([128, SQ], BF, tag="Q", bufs=2, name=f"q{P}") for P in (0, 1)]
    kp = [sb.tile([128, S], BF, tag="K", bufs=2, name=f"k{P}") for P in (0, 1)]
    vts = [None] * NPR
    emit_q(0, qp[0], 0)
    for skb in range(8):
        emit_k(0, kp[0], skb)
        for pr in (2 * skb, 2 * skb + 1):
            vts[pr] = emit_vpair(pr)
    for sqb in range(1, 4):
        emit_q(0, qp[0], sqb)
    for sqb in range(4):
        emit_q(1, qp[1], sqb)
    for skb in range(8):
        emit_k(1, kp[1], skb)

    return dict(qp=qp, kp=kp, vts=vts, x_in=x_in, pb_t=pb_t, shift_t=shift_t,
                wp0=wp0, wp1=wp1, out_d=out_d)


def _emit_attn(nc, tc, sb, ps, dr, ctx, dbg=None):
    qp, kp, vts, pb_t = ctx["qp"], ctx["kp"], ctx["vts"], ctx["pb_t"]
    shift_t, wp0, wp1, out_d = ctx["shift_t"], ctx["wp0"], ctx["wp1"], ctx["out_d"]
    x_res_dram = ctx["x_in"]

    _rr = [0]
    PS_TAGS = ["T", "O0", "O1"]
    def rr_ps(shape):
        tag = PS_TAGS[_rr[0] % 3]
        _rr[0] += 1
        return ps.tile(shape, FP, tag=tag, name=f"amm{_rr[0]}", bufs=3 if tag == "T" else 1)

    # ---------------- phase 3: attention ----------------
    o_sb = [sb.tile([128, SQ], BF, tag="MIDO", bufs=2, name=f"osb{i}") for i in (0, 1)]
    for P in (0, 1):
        for sqb in range(4):
            O_ps = [ps.tile([65, 512], FP, tag="O0", name=f"O0_{P}_{sqb}"),
                    ps.tile([65, 512], FP, tag="O1", name=f"O1_{P}_{sqb}")]
            def emit_pv(u):
                u_rhs, upr, uhg, uO = u
                nc.tensor.matmul(
                    uO, lhsT=vts[upr][:, :, uhg, 0:65], rhs=u_rhs,
                    start=(upr == 0), stop=(upr == NPR - 1), perf_mode=DR)

            # software-pipelined: emit QK+exp of unit u, then the PV of unit
            # u-1, so the PE always has work while ScalarE/DVE run exp
            pending = None
            for pr in range(NPR):
                for h in (0, 1):
                    hg = 2 * P + h
                    T = ps.tile([128, 1024], FP, tag="T", bufs=3)
                    for j in range(2):
                        ck = 2 * pr + j
                        nc.tensor.matmul(
                            T[:, j * 512:(j + 1) * 512],
                            lhsT=kp[P][64 * h:64 * (h + 1), ck * 128:(ck + 1) * 128],
                            rhs=qp[P][64 * h:64 * (h + 1), sqb * 512:(sqb + 1) * 512],
                            start=True, stop=True, tile_position=(64 * h, 0))
                    if pr in DVE_PAIRS:
                        PT = sb.tile([128, 1024], U8, tag="PTD", bufs=4)
                        nc.vector.tensor_scalar(PT, T, scalar1=TS_SCALE1,
                                                scalar2=TS_SCALE2,
                                                op0=AluOpType.mult, op1=AluOpType.add)
                        rhs_ap = PT.bitcast(F8).rearrange("p (k n) -> p k n", k=2)
                    else:
                        PT = sb.tile([128, 1024], F8, tag="PTS", bufs=4)
                        nc.scalar.activation(PT, T, AF.Exp, scale=0.125, bias=shift_t)
                        rhs_ap = PT[:, :].rearrange("p (k n) -> p k n", k=2)
                    if dbg is not None and P == 0 and sqb == 0 and pr < 2:
                        tdump = sb.tile([128, 1024], FP, tag="DBG", bufs=4,
                                        name=f"tdump{pr}{h}")
                        nc.vector.tensor_copy(tdump, T)
                        nc.sync.dma_start(out=dbg[f"dT{pr}{h}"][:, :], in_=tdump)
                        nc.sync.dma_start(out=dbg[f"dPT{pr}{h}"][:, :],
                                          in_=PT.bitcast(U8))
                    if pending is not None:
                        emit_pv(pending)
                    pending = (rhs_ap, pr, hg, O_ps[h])
            emit_pv(pending)
            pending = None
            for h in (0, 1):
                oc = sb.tile([65, 512], FP, tag="OC", bufs=2)
                nc.vector.tensor_copy(oc, O_ps[h])
                if dbg is not None and P == 0 and sqb == 0:
                    nc.sync.dma_start(out=dbg[f"doc{h}"][:, :], in_=oc)
                # broadcast the denominator row across 64 partitions: bounce
                # through DRAM (partition-stride-0 APs are DRAM-only), then
                # reciprocal on DVE
                dscr = dr.tile([1, 512], FP, tag="DSC", bufs=4, name=f"dsc{P}{sqb}{h}")
                nc.sync.dma_start(out=dscr, in_=oc[65 - 1:65, :])
                rbc = sb.tile([64, 512], FP, tag="RBC", bufs=2)
                den_bcast = bass.AP(tensor=dscr.tensor, offset=dscr.offset,
                                    ap=[[0, 64], [1, 512]])
                nc.sync.dma_start(out=rbc, in_=den_bcast)
                nc.vector.reciprocal_approx_fast(rbc, rbc)
                if h == 0:
                    nc.vector.tensor_tensor(o_sb[P][0:64, sqb * 512:(sqb + 1) * 512],
                                            oc[0:64, :], rbc, op=AluOpType.mult)
                else:
                    tmp = sb.tile([64, 512], BF, tag="TMP", bufs=2)
                    nc.vector.tensor_tensor(tmp, oc[0:64, :], rbc, op=AluOpType.mult)
                    # shift to partitions 64..127 via SBUF->SBUF DMA
                    nc.sync.dma_start(out=o_sb[P][64:128, sqb * 512:(sqb + 1) * 512], in_=tmp)

    if dbg is not None:
        nc.sync.dma_start(out=dbg["dq0"][:, :], in_=qp[0])
        nc.sync.dma_start(out=dbg["dk0"][:, :], in_=kp[0])
        nc.sync.dma_start(out=dbg["dvt0"][:, :],
                          in_=vts[0].rearrange("p a b c -> p (a b c)").bitcast(U8))
        nc.sync.dma_start(out=dbg["dosb0"][:, :], in_=o_sb[0])
        nc.sync.dma_start(out=dbg["dosb1"][:, :], in_=o_sb[1])

    # ---------------- phase 4: projection + residual ----------------
    for och in (0, 1):
        for sqb in range(4):
            pp = rr_ps([128, 512])
            nc.tensor.matmul(pp, lhsT=wp0[:, och * 128:(och + 1) * 128],
                             rhs=o_sb[0][:, sqb * 512:(sqb + 1) * 512], start=True, stop=False)
            nc.tensor.matmul(pp, lhsT=wp1[:, och * 128:(och + 1) * 128],
                             rhs=o_sb[1][:, sqb * 512:(sqb + 1) * 512], start=False, stop=True)
            fin = sb.tile([128, 512], FP, tag="FIN", bufs=4)
            xrs = sb.tile([128, 512], FP, tag="XR2", bufs=4, name=f"xr{och}{sqb}")
            nc.sync.dma_start(out=xrs, in_=x_res_dram[och * 128:(och + 1) * 128,
                                                      sqb * 512:(sqb + 1) * 512])
            nc.vector.scalar_tensor_tensor(fin, in0=pp, scalar=pb_t[och],
                                           in1=xrs,
                                           op0=AluOpType.add, op1=AluOpType.add)
            nc.sync.dma_start(out=out_d[och * 128:(och + 1) * 128, sqb * 512:(sqb + 1) * 512],
                              in_=fin)


def _emit_body(nc, tc, sb, ps, dr, dram, dbg=None):
    ctx = _emit_prefix(nc, tc, sb, ps, dr, dram)
    _emit_attn(nc, tc, sb, ps, dr, ctx, dbg=dbg)


def build_program(loop_n: int = 1, debug: bool = False, timing: bool = False, loop_part: str = "all"):
    nc = bacc.Bacc("TRN2", target_bir_lowering=False)
    if timing:
        # Timing-only build: identical device work, but all big tensors are
        # internal DRAM (garbage contents — every op here is data-independent
        # in latency), so each dispatch ships only a 4-byte token instead of
        # ~7MB/core. Wall-clock deltas then resolve the kernel time.
        tok_in = nc.dram_tensor("tok", [1, 1], FP, kind="ExternalInput")
        tok_out = nc.dram_tensor("tok_out", [1, 1], FP, kind="ExternalOutput")
        kind = "Internal"
    else:
        kind = "ExternalInput"
    x_in = nc.dram_tensor("x", [C, S], FP, kind=kind)
    wqkvT = nc.dram_tensor("wqkvT", [C, 3 * C], BF, kind=kind)
    wprojT = nc.dram_tensor("wprojT", [C, C], BF, kind=kind)
    bqk = nc.dram_tensor("bqk", [2 * C, 1], FP, kind=kind)
    gnw = nc.dram_tensor("gnw", [C, 1], FP, kind=kind)
    gnb = nc.dram_tensor("gnb", [C, 1], FP, kind=kind)
    pbeff = nc.dram_tensor("pbeff", [C, 1], FP, kind=kind)
    oneg = nc.dram_tensor("oneg", [128, 4], FP, kind=kind)
    onegT = nc.dram_tensor("onegT", [4, 128], FP, kind=kind)
    out_d = nc.dram_tensor("out", [C, SQ], FP,
                           kind="Internal" if timing else "ExternalOutput")
    dram = (x_in, wqkvT, wprojT, bqk, gnw, gnb, pbeff, oneg, onegT, out_d)

    dbg = None
    if debug:
        dbg = {}
        specs = [("doc0", [65, 512], FP), ("doc1", [65, 512], FP),
                 ("dq0", [128, SQ], BF), ("dk0", [128, S], BF),
                 ("dvt0", [128, 640], U8),
                 ("dosb0", [128, SQ], BF), ("dosb1", [128, SQ], BF)]
        for pr in range(2):
            for h in range(2):
                specs.append((f"dT{pr}{h}", [128, 1024], FP))
                specs.append((f"dPT{pr}{h}", [128, 1024], U8))
        for name, shape, dt_ in specs:
            dbg[name] = nc.dram_tensor(name, shape, dt_, kind="ExternalOutput")

    with tile.TileContext(nc) as tc:
        with (tc.tile_pool(name="sb", bufs=1) as sb,
              tc.tile_pool(name="ps", bufs=1, space="PSUM") as ps,
              tc.tile_pool(name="dr", bufs=1, space="DRAM") as dr):
            if timing:
                tokt = sb.tile([1, 1], FP, tag="tok")
                nc.sync.dma_start(out=tokt, in_=tok_in[:, :])
                nc.sync.dma_start(out=tok_out[:, :], in_=tokt)
            # psum tag slots: T 3x2 banks, O0/O1 1 bank each = 8 banks
            if loop_n == 1:
                _emit_body(nc, tc, sb, ps, dr, dram, dbg=dbg)
            elif loop_part == "all":
                with tc.For_i(0, loop_n, 1):
                    _emit_body(nc, tc, sb, ps, dr, dram)
            elif loop_part == "prefix":
                with tc.For_i(0, loop_n, 1):
                    _emit_prefix(nc, tc, sb, ps, dr, dram)
            elif loop_part == "attn":
                ctx = _emit_prefix(nc, tc, sb, ps, dr, dram)
                with tc.For_i(0, loop_n, 1):
                    _emit_attn(nc, tc, sb, ps, dr, ctx)
            else:
                raise ValueError(loop_part)
    nc.finalize()
    return nc


def make_in_maps(inputs):
    x = np.asarray(inputs["x"], np.float32).reshape(B, C, S)
    gn_w = np.asarray(inputs["gn_w"], np.float32)
    gn_b = np.asarray(inputs["gn_b"], np.float32)
    qkv_w = np.asarray(inputs["qkv_w"], np.float32)
    qkv_b = np.asarray(inputs["qkv_b"], np.float32)
    proj_w = np.asarray(inputs["proj_w"], np.float32)
    proj_b = np.asarray(inputs["proj_b"], np.float32)

    wqkvT = np.ascontiguousarray(qkv_w.T).astype(ml_dtypes.bfloat16)   # [C, 3C]
    wprojT = np.ascontiguousarray(proj_w.T).astype(ml_dtypes.bfloat16)  # [C, C]
    bqk = np.ascontiguousarray(qkv_b[:2 * C].reshape(2 * C, 1))
    bv = qkv_b[2 * C:]
    pbeff = np.ascontiguousarray((proj_w @ bv + proj_b).reshape(C, 1).astype(np.float32))
    gnw2 = np.ascontiguousarray(gn_w.reshape(C, 1))
    gnb2 = np.ascontiguousarray(gn_b.reshape(C, 1))
    oneg = np.zeros((128, 4), np.float32)
    oneg[np.arange(128), np.arange(128) // 32] = 1.0
    onegT = np.ascontiguousarray(oneg.T)

    in_maps = []
    for c in range(8):
        b, half = c // 2, c % 2
        x_rot = np.ascontiguousarray(np.roll(x[b], -half * SQ, axis=1))
        in_maps.append(dict(x=x_rot, wqkvT=wqkvT, wprojT=wprojT, bqk=bqk,
                            gnw=gnw2, gnb=gnb2, pbeff=pbeff, oneg=oneg, onegT=onegT))
    return in_maps


def assemble_output(results):
    out = np.empty((B, C, S), np.float32)
    for c in range(8):
        b, half = c // 2, c % 2
        out[b][:, half * SQ:(half + 1) * SQ] = results[c]["out"]
    return out.reshape(B, C, 64, 64)


_prog_cache = {}


def kernel(**inputs):
    if "nc" not in _prog_cache:
        _prog_cache["nc"] = build_program(loop_n=1)
    nc = _prog_cache["nc"]
    in_maps = make_in_maps(inputs)
    res = run_bass_kernel_spmd(nc, in_maps, core_ids=list(range(8)), trace=False)
    return assemble_output(res.results)


# revision 10
# speedup vs baseline: 1.9845x; 1.4640x over previous
"""Trainium2 Bass kernel for nn_AttentionBlock (GroupNorm -> QKV -> 4-head
attention over 4096 seq -> proj -> residual), B=4, C=256, H=W=64.

Sharding: 8 cores = (batch b, sequence-half). Each core redundantly computes
GroupNorm + K/V for its batch (cheap), and Q/attention/proj for its half of
the sequence. No collectives. The host rotates x's spatial columns per core
so a single SPMD program serves all 8 cores (attention output is invariant
to key/value ordering; queries are always columns 0:2048 of the rotated x).

v2 engine plan (per core, 33.5M softmax exps is the roofline):
  - QKV/QK/proj matmuls in bf16 (PE streams 1 col/cycle @2.4GHz warm; QK
    row-packs the two heads of a pair at tile_position (0,0)/(64,0)).
  - exp is SPLIT across two engines, alternating sk-chunk-pairs:
      ScalarE: true exp LUT -> fp8e4m3 PT   (153.6 G elem/s)
      VectorE: Schraudolph bit-trick -> uint8 -> bitcast fp8e4m3
               (y = round(1.4427*T + B); fp32->uint8 saturates at 0 for
               logit underflow; one 1x tensor_scalar from PSUM, 123 G/s)
    A global logit shift of -4 (exp(x-4)) keeps exp below fp8 max; it
    cancels in the softmax ratio. Both engines use the same shift.
  - PV runs in fp8 DoubleRow (2 contraction rows/cell, 0.5 cyc/row): V is
    stored as [128, (ko=2, h=4, 80pad)] fp8 pair tiles with a ones column
    at d=64 so O_ps[64,:] accumulates the softmax denominator.
  - normalize: denominator row bounced through DRAM to broadcast across
    partitions, reciprocal_approx_fast + multiply on DVE; proj in bf16 and
    residual-add in fp32 as before.
"""

import math
import numpy as np
import ml_dtypes

import concourse.bacc as bacc
import concourse.bass as bass
import concourse.tile as tile
from concourse import mybir
from concourse.alu_op_type import AluOpType
from concourse.bass_utils import run_bass_kernel_spmd

B, C, S = 4, 256, 4096
NH, D, G = 4, 64, 8
EPS = 1e-5
SQ = S // 2          # 2048 queries per core
NCK = S // 128       # 32 key chunks
NPR = NCK // 2       # 16 chunk pairs (DoubleRow contraction = 256)
FP = mybir.dt.float32
BF = mybir.dt.bfloat16
F8 = mybir.dt.float8e4
U8 = mybir.dt.uint8
AF = mybir.ActivationFunctionType
DR = mybir.MatmulPerfMode.DoubleRow

# Schraudolph bf16 trick: int16 = round(A16*(0.125*T) + B16), bitcast bf16
A16 = 128.0 / math.log(2.0)       # 184.664 (128 = 2^mantissa_bits)
TS_SCALE1 = 0.125 * A16           # 23.0831
TS_SCALE2 = 127 * 128 - 0.558 * 128 / 8   # 16247.07 (exp bias 127, mid adjust)

# pairs whose exp runs on VectorE (rest on ScalarE): 7/16 = 43.75%
DVE_PAIRS = frozenset({1, 3, 5, 7, 9, 11, 13})


def _emit_prefix(nc, tc, sb, ps, dr, dram):
    x_in, wqkvT, wprojT, bqk, gnw, gnb, pbeff, oneg, onegT, out_d = dram

    # ---------------- constants / weights ----------------
    c_oneg = sb.tile([128, 4], FP, tag="cg")
    nc.sync.dma_start(out=c_oneg, in_=oneg[:, :])
    c_onegT = sb.tile([4, 128], FP, tag="cgt")
    nc.sync.dma_start(out=c_onegT, in_=onegT[:, :])

    w0 = sb.tile([128, 3 * C], BF, tag="W", bufs=2)
    w1 = sb.tile([128, 3 * C], BF, tag="W", bufs=2)
    nc.sync.dma_start(out=w0, in_=wqkvT[0:128, :])
    nc.sync.dma_start(out=w1, in_=wqkvT[128:256, :])
    wp0 = sb.tile([128, C], BF, tag="WP", bufs=2)
    wp1 = sb.tile([128, C], BF, tag="WP", bufs=2)
    nc.sync.dma_start(out=wp0, in_=wprojT[0:128, :])
    nc.sync.dma_start(out=wp1, in_=wprojT[128:256, :])

    def vec(name):
        return sb.tile([128, 1], FP, tag="vecs", bufs=12, name=name)

    gnw_t, gnb_t, pb_t, bq_t, bk_t = [], [], [], [], []
    for t in (0, 1):
        gw = vec(f"gnw{t}"); nc.sync.dma_start(out=gw, in_=gnw[t * 128:(t + 1) * 128, :]); gnw_t.append(gw)
        gb = vec(f"gnb{t}"); nc.sync.dma_start(out=gb, in_=gnb[t * 128:(t + 1) * 128, :]); gnb_t.append(gb)
        pb = vec(f"pbe{t}"); nc.sync.dma_start(out=pb, in_=pbeff[t * 128:(t + 1) * 128, :]); pb_t.append(pb)
        bq = vec(f"bq{t}"); nc.sync.dma_start(out=bq, in_=bqk[t * 128:(t + 1) * 128, :]); bq_t.append(bq)
        bk = vec(f"bk{t}"); nc.sync.dma_start(out=bk, in_=bqk[256 + t * 128:256 + (t + 1) * 128, :]); bk_t.append(bk)


    # round-robin psum tags for the matmul-evac phases (pre/post attention)
    _rr = [0]
    PS_TAGS = ["T", "O0", "O1"]
    def rr_ps(shape):
        tag = PS_TAGS[_rr[0] % 3]
        _rr[0] += 1
        return ps.tile(shape, FP, tag=tag, name=f"mm{_rr[0]}", bufs=3 if tag == "T" else 1)

    # ---------------- phase 1: load x + GroupNorm ----------------
    xns = []
    for t in (0, 1):
        x_t = sb.tile([128, S], FP, tag="BIG", bufs=2)
        for c8 in range(8):
            nc.sync.dma_start(out=x_t[:, c8 * 512:(c8 + 1) * 512],
                              in_=x_in[t * 128:(t + 1) * 128, c8 * 512:(c8 + 1) * 512])
        st = sb.tile([128, 8, 6], FP, tag="bn", bufs=2)
        for c8 in range(8):
            nc.vector.bn_stats(out=st[:, c8, :], in_=x_t[:, c8 * 512:(c8 + 1) * 512])
        mv = sb.tile([128, 2], FP, tag="mv", bufs=2)
        nc.vector.bn_aggr(out=mv, in_=st)
        # per-partition [mean, E[x^2]]
        s2 = sb.tile([128, 2], FP, tag="s2", bufs=2)
        nc.vector.tensor_copy(s2[:, 0:1], mv[:, 0:1])
        nc.vector.tensor_tensor(s2[:, 1:2], mv[:, 0:1], mv[:, 0:1], op=AluOpType.mult)
        nc.vector.tensor_tensor(s2[:, 1:2], s2[:, 1:2], mv[:, 1:2], op=AluOpType.add)
        # reduce over the 32 partitions of each group (4 groups per 128-chunk)
        gps = rr_ps([4, 2])
        nc.tensor.matmul(gps, lhsT=c_oneg, rhs=s2, start=True, stop=True)
        gsb = sb.tile([4, 2], FP, tag="gsb", bufs=2)
        nc.vector.tensor_scalar_mul(gsb, gps, 1.0 / 32.0)
        tmp4 = sb.tile([4, 1], FP, tag="t4", bufs=2)
        nc.vector.tensor_tensor(tmp4, gsb[:, 0:1], gsb[:, 0:1], op=AluOpType.mult)
        nc.vector.tensor_tensor(gsb[:, 1:2], gsb[:, 1:2], tmp4, op=AluOpType.subtract)
        epsb = sb.tile([4, 1], FP, tag="eps", bufs=2, name=f"eps{t}")
        nc.vector.memset(epsb, EPS)
        nc.scalar.activation(gsb[:, 1:2], gsb[:, 1:2], AF.Sqrt, bias=epsb)
        nc.vector.reciprocal(gsb[:, 1:2], gsb[:, 1:2])       # rstd = 1/sqrt(var+eps)
        # broadcast group stats back to the 128 channel partitions
        bc = rr_ps([128, 2])
        nc.tensor.matmul(bc, lhsT=c_onegT, rhs=gsb, start=True, stop=True)
        scl = sb.tile([128, 1], FP, tag="scl", bufs=2)
        sft = sb.tile([128, 1], FP, tag="sft", bufs=2)
        nc.vector.tensor_tensor(scl, bc[:, 1:2], gnw_t[t], op=AluOpType.mult)
        nc.vector.tensor_tensor(sft, bc[:, 0:1], scl, op=AluOpType.mult)
        nc.vector.tensor_tensor(sft, gnb_t[t], sft, op=AluOpType.subtract)
        xn_t = sb.tile([128, S], BF, tag="MID", bufs=2)
        nc.vector.tensor_scalar(xn_t, x_t, scalar1=scl, scalar2=sft,
                                op0=AluOpType.mult, op1=AluOpType.add)
        xns.append(xn_t)

    # ---------------- phase 2: QKV projections ----------------
    # q[qch, sq] for this core's queries; k[kch, sk] full (bf16);
    # v as fp8 DoubleRow pair tiles [128, (ko=2, h=4, 80)] with ones col at 64
    def emit_q(P, q_t, sqb):
        mm = rr_ps([128, 512])
        nc.tensor.matmul(mm, lhsT=w0[:, P * 128:(P + 1) * 128],
                         rhs=xns[0][:, sqb * 512:(sqb + 1) * 512], start=True, stop=False)
        nc.tensor.matmul(mm, lhsT=w1[:, P * 128:(P + 1) * 128],
                         rhs=xns[1][:, sqb * 512:(sqb + 1) * 512], start=False, stop=True)
        nc.scalar.activation(q_t[:, sqb * 512:(sqb + 1) * 512], mm, AF.Identity, bias=bq_t[P])

    def emit_k(P, k_t, skb):
        mm = rr_ps([128, 512])
        nc.tensor.matmul(mm, lhsT=w0[:, 256 + P * 128:256 + (P + 1) * 128],
                         rhs=xns[0][:, skb * 512:(skb + 1) * 512], start=True, stop=False)
        nc.tensor.matmul(mm, lhsT=w1[:, 256 + P * 128:256 + (P + 1) * 128],
                         rhs=xns[1][:, skb * 512:(skb + 1) * 512], start=False, stop=True)
        nc.scalar.activation(k_t[:, skb * 512:(skb + 1) * 512], mm, AF.Identity, bias=bk_t[P])

    def emit_vpair(pr):
        # two sk chunks c0=2pr, c1=2pr+1 -> one [128, 512] psum ([ko, h, d])
        mm = rr_ps([128, 512])
        for j in range(2):
            ck = 2 * pr + j
            nc.tensor.matmul(mm[:, j * 256:(j + 1) * 256],
                             lhsT=xns[0][:, ck * 128:(ck + 1) * 128],
                             rhs=w0[:, 512:768], start=True, stop=False)
            nc.tensor.matmul(mm[:, j * 256:(j + 1) * 256],
                             lhsT=xns[1][:, ck * 128:(ck + 1) * 128],
                             rhs=w1[:, 512:768], start=False, stop=True)
        vt = sb.tile([128, 2, 4, 65], BF, tag="VT", bufs=NPR, name=f"vt{pr}")
        nc.scalar.activation(vt[:, :, :, 0:64],
                             mm.rearrange("p (k h u) -> p k h u", k=2, u=64),
                             AF.Identity)
        nc.vector.memset(vt[:, :, :, 64:65], 1.0)
        return vt

    # emission order: unblock pair-0 attention ASAP (q P0 + interleaved k P0/vt),
    # then pair-1 q/k (not needed until ~half-way through attention)
    qp = [sb.tile([128, SQ], BF, tag="Q", bufs=2, name=f"q{P}") for P in (0, 1)]
    kp = [sb.tile([128, S], BF, tag="K", bufs=2, name=f"k{P}") for P in (0, 1)]
    vts = [None] * NPR
    emit_q(0, qp[0], 0)
    for skb in range(8):
        emit_k(0, kp[0], skb)
        for pr in (2 * skb, 2 * skb + 1):
            vts[pr] = emit_vpair(pr)
    for sqb in range(1, 4):
        emit_q(0, qp[0], sqb)
    for sqb in range(4):
        emit_q(1, qp[1], sqb)
    for skb in range(8):
        emit_k(1, kp[1], skb)

    return dict(qp=qp, kp=kp, vts=vts, x_in=x_in, pb_t=pb_t,
                wp0=wp0, wp1=wp1, out_d=out_d)


def _emit_attn(nc, tc, sb, ps, dr, ctx, dbg=None):
    qp, kp, vts, pb_t = ctx["qp"], ctx["kp"], ctx["vts"], ctx["pb_t"]
    wp0, wp1, out_d = ctx["wp0"], ctx["wp1"], ctx["out_d"]
    x_res_dram = ctx["x_in"]

    _rr = [0]
    PS_TAGS = ["T", "O0", "O1"]
    def rr_ps(shape):
        tag = PS_TAGS[_rr[0] % 3]
        _rr[0] += 1
        return ps.tile(shape, FP, tag=tag, name=f"amm{_rr[0]}", bufs=3 if tag == "T" else 1)

    # ---------------- phase 3: attention ----------------
    o_sb = [sb.tile([128, SQ], BF, tag="MIDO", bufs=2, name=f"osb{i}") for i in (0, 1)]
    for P in (0, 1):
        for sqb in range(4):
            O_ps = [ps.tile([65, 512], FP, tag="O0", name=f"O0_{P}_{sqb}"),
                    ps.tile([65, 512], FP, tag="O1", name=f"O1_{P}_{sqb}")]
            def emit_pv(u):
                u_rhs, upr, uhg, uO = u
                for j in range(2):
                    ck = 2 * upr + j
                    nc.tensor.matmul(
                        uO, lhsT=vts[upr][:, j, uhg, :],
                        rhs=u_rhs[:, j * 512:(j + 1) * 512],
                        start=(ck == 0), stop=(ck == NCK - 1))

            # software-pipelined: emit QK+exp of unit u, then the PV of unit
            # u-1, so the PE always has work while ScalarE/DVE run exp
            pending = None
            for pr in range(NPR):
                for h in (0, 1):
                    hg = 2 * P + h
                    T = ps.tile([128, 1024], FP, tag="T", bufs=3)
                    for j in range(2):
                        ck = 2 * pr + j
                        nc.tensor.matmul(
                            T[:, j * 512:(j + 1) * 512],
                            lhsT=kp[P][64 * h:64 * (h + 1), ck * 128:(ck + 1) * 128],
                            rhs=qp[P][64 * h:64 * (h + 1), sqb * 512:(sqb + 1) * 512],
                            start=True, stop=True, tile_position=(64 * h, 0))
                    if pr in DVE_PAIRS:
                        PT = sb.tile([128, 1024], mybir.dt.int16, tag="PTD", bufs=4)
                        nc.vector.tensor_scalar(PT, T, scalar1=TS_SCALE1,
                                                scalar2=TS_SCALE2,
                                                op0=AluOpType.mult, op1=AluOpType.add)
                        rhs_ap = PT.bitcast(BF)
                    else:
                        PT = sb.tile([128, 1024], BF, tag="PTS", bufs=4)
                        nc.scalar.activation(PT, T, AF.Exp, scale=0.125)
                        rhs_ap = PT[:, :]
                    if dbg is not None and P == 0 and sqb == 0 and pr < 2:
                        tdump = sb.tile([128, 1024], FP, tag="DBG", bufs=4,
                                        name=f"tdump{pr}{h}")
                        nc.vector.tensor_copy(tdump, T)
                        nc.sync.dma_start(out=dbg[f"dT{pr}{h}"][:, :], in_=tdump)
                        nc.sync.dma_start(out=dbg[f"dPT{pr}{h}"][:, :],
                                          in_=PT.bitcast(U8))
                    if pending is not None:
                        emit_pv(pending)
                    pending = (rhs_ap, pr, hg, O_ps[h])
            emit_pv(pending)
            pending = None
            for h in (0, 1):
                oc = sb.tile([65, 512], FP, tag="OC", bufs=2)
                nc.vector.tensor_copy(oc, O_ps[h])
                if dbg is not None and P == 0 and sqb == 0:
                    nc.sync.dma_start(out=dbg[f"doc{h}"][:, :], in_=oc)
                # broadcast the denominator row across 64 partitions: bounce
                # through DRAM (partition-stride-0 APs are DRAM-only), then
                # reciprocal on DVE
                dscr = dr.tile([1, 512], FP, tag="DSC", bufs=4, name=f"dsc{P}{sqb}{h}")
                nc.sync.dma_start(out=dscr, in_=oc[65 - 1:65, :])
                rbc = sb.tile([64, 512], FP, tag="RBC", bufs=2)
                den_bcast = bass.AP(tensor=dscr.tensor, offset=dscr.offset,
                                    ap=[[0, 64], [1, 512]])
                nc.sync.dma_start(out=rbc, in_=den_bcast)
                nc.vector.reciprocal_approx_fast(rbc, rbc)
                if h == 0:
                    nc.vector.tensor_tensor(o_sb[P][0:64, sqb * 512:(sqb + 1) * 512],
                                            oc[0:64, :], rbc, op=AluOpType.mult)
                else:
                    tmp = sb.tile([64, 512], BF, tag="TMP", bufs=2)
                    nc.vector.tensor_tensor(tmp, oc[0:64, :], rbc, op=AluOpType.mult)
                    # shift to partitions 64..127 via SBUF->SBUF DMA
                    nc.sync.dma_start(out=o_sb[P][64:128, sqb * 512:(sqb + 1) * 512], in_=tmp)

    if dbg is not None:
        nc.sync.dma_start(out=dbg["dq0"][:, :], in_=qp[0])
        nc.sync.dma_start(out=dbg["dk0"][:, :], in_=kp[0])
        nc.sync.dma_start(out=dbg["dvt0"][:, :],
                          in_=vts[0].rearrange("p a b c -> p (a b c)").bitcast(U8))
        nc.sync.dma_start(out=dbg["dosb0"][:, :], in_=o_sb[0])
        nc.sync.dma_start(out=dbg["dosb1"][:, :], in_=o_sb[1])

    # ---------------- phase 4: projection + residual ----------------
    for och in (0, 1):
        for sqb in range(4):
            pp = rr_ps([128, 512])
            nc.tensor.matmul(pp, lhsT=wp0[:, och * 128:(och + 1) * 128],
                             rhs=o_sb[0][:, sqb * 512:(sqb + 1) * 512], start=True, stop=False)
            nc.tensor.matmul(pp, lhsT=wp1[:, och * 128:(och + 1) * 128],
                             rhs=o_sb[1][:, sqb * 512:(sqb + 1) * 512], start=False, stop=True)
            fin = sb.tile([128, 512], FP, tag="FIN", bufs=4)
            xrs = sb.tile([128, 512], FP, tag="XR2", bufs=4, name=f"xr{och}{sqb}")
            nc.sync.dma_start(out=xrs, in_=x_res_dram[och * 128:(och + 1) * 128,
                                                      sqb * 512:(sqb + 1) * 512])
            nc.vector.scalar_tensor_tensor(fin, in0=pp, scalar=pb_t[och],
                                           in1=xrs,
                                           op0=AluOpType.add, op1=AluOpType.add)
            nc.sync.dma_start(out=out_d[och * 128:(och + 1) * 128, sqb * 512:(sqb + 1) * 512],
                              in_=fin)


def _emit_body(nc, tc, sb, ps, dr, dram, dbg=None):
    ctx = _emit_prefix(nc, tc, sb, ps, dr, dram)
    _emit_attn(nc, tc, sb, ps, dr, ctx, dbg=dbg)


def build_program(loop_n: int = 1, debug: bool = False, timing: bool = False, loop_part: str = "all"):
    nc = bacc.Bacc("TRN2", target_bir_lowering=False)
    if timing:
        # Timing-only build: identical device work, but all big tensors are
        # internal DRAM (garbage contents — every op here is data-independent
        # in latency), so each dispatch ships only a 4-byte token instead of
        # ~7MB/core. Wall-clock deltas then resolve the kernel time.
        tok_in = nc.dram_tensor("tok", [1, 1], FP, kind="ExternalInput")
        tok_out = nc.dram_tensor("tok_out", [1, 1], FP, kind="ExternalOutput")
        kind = "Internal"
    else:
        kind = "ExternalInput"
    x_in = nc.dram_tensor("x", [C, S], FP, kind=kind)
    wqkvT = nc.dram_tensor("wqkvT", [C, 3 * C], BF, kind=kind)
    wprojT = nc.dram_tensor("wprojT", [C, C], BF, kind=kind)
    bqk = nc.dram_tensor("bqk", [2 * C, 1], FP, kind=kind)
    gnw = nc.dram_tensor("gnw", [C, 1], FP, kind=kind)
    gnb = nc.dram_tensor("gnb", [C, 1], FP, kind=kind)
    pbeff = nc.dram_tensor("pbeff", [C, 1], FP, kind=kind)
    oneg = nc.dram_tensor("oneg", [128, 4], FP, kind=kind)
    onegT = nc.dram_tensor("onegT", [4, 128], FP, kind=kind)
    out_d = nc.dram_tensor("out", [C, SQ], FP,
                           kind="Internal" if timing else "ExternalOutput")
    dram = (x_in, wqkvT, wprojT, bqk, gnw, gnb, pbeff, oneg, onegT, out_d)

    dbg = None
    if debug:
        dbg = {}
        specs = [("doc0", [65, 512], FP), ("doc1", [65, 512], FP),
                 ("dq0", [128, SQ], BF), ("dk0", [128, S], BF),
                 ("dvt0", [128, 640], U8),
                 ("dosb0", [128, SQ], BF), ("dosb1", [128, SQ], BF)]
        for pr in range(2):
            for h in range(2):
                specs.append((f"dT{pr}{h}", [128, 1024], FP))
                specs.append((f"dPT{pr}{h}", [128, 2048], U8))
        for name, shape, dt_ in specs:
            dbg[name] = nc.dram_tensor(name, shape, dt_, kind="ExternalOutput")

    with tile.TileContext(nc) as tc:
        with (tc.tile_pool(name="sb", bufs=1) as sb,
              tc.tile_pool(name="ps", bufs=1, space="PSUM") as ps,
              tc.tile_pool(name="dr", bufs=1, space="DRAM") as dr):
            if timing:
                tokt = sb.tile([1, 1], FP, tag="tok")
                nc.sync.dma_start(out=tokt, in_=tok_in[:, :])
                nc.sync.dma_start(out=tok_out[:, :], in_=tokt)
            # psum tag slots: T 3x2 banks, O0/O1 1 bank each = 8 banks
            if loop_n == 1:
                _emit_body(nc, tc, sb, ps, dr, dram, dbg=dbg)
            elif loop_part == "all":
                with tc.For_i(0, loop_n, 1):
                    _emit_body(nc, tc, sb, ps, dr, dram)
            elif loop_part == "prefix":
                with tc.For_i(0, loop_n, 1):
                    _emit_prefix(nc, tc, sb, ps, dr, dram)
            elif loop_part == "attn":
                ctx = _emit_prefix(nc, tc, sb, ps, dr, dram)
                with tc.For_i(0, loop_n, 1):
                    _emit_attn(nc, tc, sb, ps, dr, ctx)
            else:
                raise ValueError(loop_part)
    nc.finalize()
    return nc


def make_in_maps(inputs):
    x = np.asarray(inputs["x"], np.float32).reshape(B, C, S)
    gn_w = np.asarray(inputs["gn_w"], np.float32)
    gn_b = np.asarray(inputs["gn_b"], np.float32)
    qkv_w = np.asarray(inputs["qkv_w"], np.float32)
    qkv_b = np.asarray(inputs["qkv_b"], np.float32)
    proj_w = np.asarray(inputs["proj_w"], np.float32)
    proj_b = np.asarray(inputs["proj_b"], np.float32)

    wqkvT = np.ascontiguousarray(qkv_w.T).astype(ml_dtypes.bfloat16)   # [C, 3C]
    wprojT = np.ascontiguousarray(proj_w.T).astype(ml_dtypes.bfloat16)  # [C, C]
    bqk = np.ascontiguousarray(qkv_b[:2 * C].reshape(2 * C, 1))
    bv = qkv_b[2 * C:]
    pbeff = np.ascontiguousarray((proj_w @ bv + proj_b).reshape(C, 1).astype(np.float32))
    gnw2 = np.ascontiguousarray(gn_w.reshape(C, 1))
    gnb2 = np.ascontiguousarray(gn_b.reshape(C, 1))
    oneg = np.zeros((128, 4), np.float32)
    oneg[np.arange(128), np.arange(128) // 32] = 1.0
    onegT = np.ascontiguousarray(oneg.T)

    in_maps = []
    for c in range(8):
        b, half = c // 2, c % 2
        x_rot = np.ascontiguousarray(np.roll(x[b], -half * SQ, axis=1))
        in_maps.append(dict(x=x_rot, wqkvT=wqkvT, wprojT=wprojT, bqk=bqk,
                            gnw=gnw2, gnb=gnb2, pbeff=pbeff, oneg=oneg, onegT=onegT))
    return in_maps


def assemble_output(results):
    out = np.empty((B, C, S), np.float32)
    for c in range(8):
        b, half = c // 2, c % 2
        out[b][:, half * SQ:(half + 1) * SQ] = results[c]["out"]
    return out.reshape(B, C, 64, 64)


_prog_cache = {}


def kernel(**inputs):
    if "nc" not in _prog_cache:
        _prog_cache["nc"] = build_program(loop_n=1)
    nc = _prog_cache["nc"]
    in_maps = make_in_maps(inputs)
    res = run_bass_kernel_spmd(nc, in_maps, core_ids=list(range(8)), trace=False)
    return assemble_output(res.results)
